# revision 20
# baseline (speedup 1.0000x reference)
"""Attentional Factorization Machine kernel for 8 Trainium2 NeuronCores.

Data-parallel over batch: 1024 rows -> 128 per core. Per core, all field-pair
products hp are built on DVE via a cyclic-delta enumeration (fp16, 2x mode),
the attention MLP runs on the PE (fp16 streams, fp32 PSUM), relu is split
between ACT and DVE, and per-pair scores are accumulated per batch row with
one-hot stationary matmuls.  The p_w projection g = <hp, p_w> is NOT computed
by streaming hp through the PE again; instead per-row Gram matrices
G_k = (x*p_w)^T x are computed on the PE (59-wrapped moving tensor), bounced
through DRAM, and the pair diagonals are gathered back with a single
diagonal-stride DMA into [row, pair] layout.  Softmax + combine on-chip.
"""
import sys
for _p in ("/opt/trn_rl_repo",):
    if _p not in sys.path:
        sys.path.insert(0, _p)

import numpy as np

import concourse.bass as bass
import concourse.bacc as bacc
import concourse.mybir as mybir
import concourse.tile as tile

F32 = mybir.dt.float32
F16 = mybir.dt.float16
AF = mybir.ActivationFunctionType
ALU = mybir.AluOpType
AXIS = mybir.AxisListType

FLD = 40
NDELTA = 20
P = 780
PG = 800           # padded pair axis (i-major g layout incl. 20 dup cols)
HALF = 390
SC0 = 400          # sc half 0: deltas 1..10 (d-major cols 0..399)
SC1 = 380          # sc half 1: deltas 11..20
GW = 59            # Gram moving width (fields 0..39 + wrap 0..18)
DVE_RELU_EVERY = 6  # every Nth row's relu runs on DVE instead of ACT


def build(nc, B_c=128, blocks=(8, 8, 16, 32, 32, 32)):
    assert B_c == 128 and sum(blocks) == 128
    assert all(nb % 8 == 0 for nb in blocks)

    xTa_d = nc.dram_tensor("xTa", [128, B_c, 60], F16, kind="ExternalInput").ap()
    xTb_d = nc.dram_tensor("xTb", [128, B_c, 60], F16, kind="ExternalInput").ap()
    xtil_d = nc.dram_tensor("xtil", [128, B_c, FLD], F16, kind="ExternalInput").ap()
    wT_d = nc.dram_tensor("wT", [128, 128], F16, kind="ExternalInput").ap()
    bias_d = nc.dram_tensor("bias", [128, 1], F32, kind="ExternalInput").ap()
    negb_d = nc.dram_tensor("negb", [128, 1], F32, kind="ExternalInput").ap()
    Zh_d = nc.dram_tensor("Zh", [128, 64], F16, kind="ExternalInput").ap()
    pb_d = nc.dram_tensor("pb", [128, 1], F32, kind="ExternalInput").ap()
    out_d = nc.dram_tensor("out", [B_c, 1], F32, kind="ExternalOutput").ap()

    with tile.TileContext(nc) as tc:
        with (
            tc.tile_pool(name="const", bufs=1) as cpool,
            tc.tile_pool(name="hp", bufs=2) as hpool,
            tc.tile_pool(name="relu", bufs=4) as rpool,
            tc.tile_pool(name="awps", bufs=2, space="PSUM") as awpool,
            tc.tile_pool(name="accps", bufs=1, space="PSUM") as accpool,
            tc.tile_pool(name="gps", bufs=2, space="PSUM") as gpool,
            tc.tile_pool(name="gsb", bufs=2) as gsbpool,
            tc.tile_pool(name="gdram", bufs=1, space="DRAM") as dpool,
        ):
            wT_s = cpool.tile([128, 128], F16, tag="wT")
            bias_s = cpool.tile([128, 1], F32, tag="bias")
            negb_s = cpool.tile([128, 1], F32, tag="negb")
            Zh_s = cpool.tile([128, 64], F16, tag="Zh")
            pb_s = cpool.tile([128, 1], F32, tag="pb")
            nc.sync.dma_start(wT_s[:], wT_d[:])
            nc.sync.dma_start(bias_s[:], bias_d[:])
            nc.sync.dma_start(negb_s[:], negb_d[:])
            nc.sync.dma_start(Zh_s[:], Zh_d[:])
            nc.sync.dma_start(pb_s[:], pb_d[:])

            xTa = cpool.tile([128, B_c, 60], F16, tag="xTa")
            xTb = cpool.tile([128, B_c, 60], F16, tag="xTb")
            xtil = cpool.tile([128, B_c, FLD], F16, tag="xtil")
            sc_h0 = accpool.tile([128, 512], F32, tag="sc_h0")
            sc_h1 = accpool.tile([128, 512], F32, tag="sc_h1")
            sc_h = [sc_h0, sc_h1]

            # ping-pong DRAM halves: A holds waves 0..7 (b%32 < 16), B
            # holds waves 8..15.  Splitting lets the first diagonal gather
            # run mid-kernel with no WAR coupling to later wave stores.
            # 41 rows: dup-col diagonal reads (d=20, i=39) run one element
            # past row 39 into the next b slot (or the spare row).
            G_dramA = dpool.tile([41, B_c, GW], F16, tag="gdramA")
            G_dramB = dpool.tile([41, B_c, GW], F16, tag="gdramB")
            g_sb = cpool.tile([128, PG], F16, tag="g_sb")
            # pre-zero the overflow landing slots (never written otherwise)
            zc = cpool.tile([1, 4], F16, tag="zc")
            nc.vector.memset(zc[:], 0.0)

            def zero_cells(gd, cc0, n):
                dv = gd[:].copy()
                VecPair = type(dv.ap)
                dv.ap = VecPair([[1, 1], [32 * GW, n]])
                dv.offset = dv.offset + 39 * B_c * GW + cc0 * GW
                nc.gpsimd.dma_start(dv, zc[0:1, 0:n])

            zero_cells(G_dramA, 16, 4)          # A[39][32j+16][0]
            zero_cells(G_dramB, 32, 3)          # B[39][{32,64,96}][0]
            nc.gpsimd.dma_start(G_dramB[40:41, 0:1, 0:1], zc[0:1, 0:1])

            def diag_gather(half):
                # g_sb[b, i*20+(d-1)] = G[i, b, i+d] for b%32 in the half's
                # cc range; one DMA per 32-partition quadrant j.  Half 0 runs
                # in the background mid-kernel on the idle GpSimd queue; the
                # end-of-kernel half is spread across four queues so its
                # serial time is ~1/4.
                gd = G_dramA if half == 0 else G_dramB
                embs = [nc.gpsimd] * 4
                for j in range(4):
                    b0 = 32 * j + 16 * half
                    gdiag = gd[:].copy()
                    VecPair = type(gdiag.ap)
                    gdiag.ap = VecPair([[GW, 16], [B_c * GW + 1, FLD],
                                        [1, NDELTA]])
                    gdiag.offset = gdiag.offset + 1 + b0 * GW
                    g_dst = g_sb[b0:b0 + 16].rearrange(
                        "k (i d) -> k i d", d=NDELTA)
                    embs[j].dma_start(g_dst, gdiag)

            # depth-2 software pipeline across row pairs:
            #   stage A: mm1   stage B: relu   stage C: scores + Gram
            pending = []     # (hp3, bs, kbs) awaiting mm1
            relu_q = []      # (hp3, bs, kbs, aws) awaiting relu
            sc_q = []        # (bs, kbs, relus) awaiting scores/gram
            gstate = {"tile": None}

            def do_mm1(item):
                hp3, bs, kbs = item
                ks = [k for k, b in kbs]
                aws = []
                nmm = 0
                for k in ks:
                    aw = awpool.tile([128, 1024], F32, tag="aw", name="aw")
                    for h in (0, 1):
                        bi = nc.tensor.matmul(
                            aw[:, 512 * h:512 * h + HALF],
                            wT_s[:],
                            hp3[:, k, h * HALF:(h + 1) * HALF],
                            start=True, stop=True,
                        )
                        if nmm > 0:
                            bi.ins.ldweights = False
                        nmm += 1
                    aws.append(aw)
                relu_q.append((hp3, bs, kbs, aws))

            def do_relu(item):
                hp3, bs, kbs, aws = item
                relus = []
                for (k, b), aw in zip(kbs, aws):
                    kg = bs + k
                    relu = rpool.tile([128, P], F16, tag="relu", name="relu")
                    aw_v = aw[:].rearrange("a (u q) -> a u q", q=512)[:, :, 0:HALF]
                    relu_v = relu[:].rearrange("a (u q) -> a u q", q=HALF)
                    if kg % DVE_RELU_EVERY == 2:
                        nc.vector.tensor_scalar(
                            relu_v, aw_v, negb_s[:], bias_s[:],
                            ALU.max, ALU.add,
                        )
                    else:
                        nc.scalar.activation(relu_v, aw_v, AF.Relu,
                                             bias=bias_s[:])
                    relus.append(relu)
                sc_q.append((bs, kbs, relus))

            def do_gram(kg):
                s = kg % 8
                if s == 0:
                    gstate["tile"] = gpool.tile([128, 512], F32, tag="gt",
                                                name="gt")
                gt = gstate["tile"]
                nc.tensor.matmul(
                    gt[0:40, GW * s:GW * s + GW],
                    xtil[:, kg, :],
                    xTa[:, kg, 0:GW],
                    start=True, stop=True,
                )
                if s == 7:
                    w = kg // 8
                    gd = G_dramA if w < 8 else G_dramB
                    gsb = gsbpool.tile([40, 8 * GW], F16, tag="gsb",
                                       name="gsb")
                    if w % 2 == 0:
                        nc.scalar.copy(gsb[:], gt[0:40, 0:8 * GW])
                    else:
                        nc.vector.tensor_copy(gsb[:], gt[0:40, 0:8 * GW])
                    # slots s=4c+j hold row b = 32*j + 2*w + c; store the
                    # 4 j-slots of each c with one 3-dim DMA
                    for c in (0, 1):
                        src = gsb[:, 4 * c * GW:(4 * c + 4) * GW]
                        dv = gd[0:40].copy()
                        VecPair = type(dv.ap)
                        dv.ap = VecPair([[B_c * GW, 40], [32 * GW, 4], [1, GW]])
                        dv.offset = dv.offset + (2 * w + c) * GW
                        nc.sync.dma_start(dv, src)
                    if w == 7:
                        diag_gather(0)
                    elif w == 15:
                        diag_gather(1)

            def do_sc(item):
                bs, kbs, relus = item
                # interleave one-hot scores MMs over banks/col groups; Gram
                # MMs act as spacers (distinct PSUM bank + stationary)
                for ki, ((k, b), relu) in enumerate(zip(kbs, relus)):
                    j, mp = b // 32, b % 32
                    for hi, h in enumerate((ki & 1, 1 - (ki & 1))):
                        off, ln = (0, SC0) if h == 0 else (SC0, SC1)
                        bi = nc.tensor.matmul(
                            sc_h[h][32 * j:32 * j + 32, 0:ln],
                            Zh_s[:, 32 - mp:64 - mp],
                            relu[:, off:off + ln],
                            start=(mp == 0), stop=(mp == 31),
                            tile_position=(0, 32 * j),
                            skip_group_check=True,
                        )
                        if hi == 1:
                            # same Zh window as the previous matmul
                            bi.ins.ldweights = False
                    # Gram matmul spaces the next k's bank-adjacent pair
                    do_gram(bs + k)

            def step():
                if pending:
                    do_mm1(pending.pop(0))
                if len(relu_q) >= 2:
                    do_relu(relu_q.pop(0))
                if len(sc_q) >= 2:
                    do_sc(sc_q.pop(0))

            def flush():
                while pending or relu_q or sc_q:
                    if pending:
                        do_mm1(pending.pop(0))
                    if relu_q:
                        do_relu(relu_q.pop(0))
                    if sc_q:
                        do_sc(sc_q.pop(0))

            grp_count = [0, 0, 0, 0]
            bs = 0
            NBMAX = max(blocks)
            for t, NB in enumerate(blocks):
                nc.sync.dma_start(xTa[:, bs:bs + NB, :],
                                  xTa_d[:, bs:bs + NB, :])
                nc.sync.dma_start(xTb[:, bs:bs + NB, :],
                                  xTb_d[:, bs:bs + NB, :])
                nc.sync.dma_start(xtil[:, bs:bs + NB, :],
                                  xtil_d[:, bs:bs + NB, :])

                hp = hpool.tile([128, NBMAX * P], F16, tag="hp")
                hp3 = hp[:].rearrange("e (b q) -> e b q", q=P)

                for d in range(1, NDELTA + 1):
                    cnt = FLD if d < NDELTA else NDELTA
                    col0 = (d - 1) * FLD
                    # keep both operands 4B-aligned so DVE 2x_1P engages:
                    # even d reads xTa at offset d, odd d reads xTb at d-1
                    if d % 2 == 0:
                        in1 = xTa[:, bs:bs + NB, d:d + cnt]
                    else:
                        in1 = xTb[:, bs:bs + NB, d - 1:d - 1 + cnt]
                    nc.vector.tensor_mul(
                        hp3[:, 0:NB, col0:col0 + cnt],
                        xTa[:, bs:bs + NB, 0:cnt],
                        in1,
                    )

                kbs_all = []
                for k in range(NB):
                    j = k % 4
                    b = 32 * j + grp_count[j]
                    grp_count[j] += 1
                    kbs_all.append((k, b))
                for pi in range(0, NB, 2):
                    pending.append((hp3, bs, kbs_all[pi:pi + 2]))
                    step()
                bs += NB

            flush()

            # ---- softmax tail ----
            exp_s = cpool.tile([128, PG], F32, tag="exp_s")
            junk = cpool.tile([128, PG], F16, tag="junk")
            denom = cpool.tile([128, 1], F32, tag="denom")
            rden = cpool.tile([128, 1], F32, tag="rden")
            numer = cpool.tile([128, 1], F32, tag="numer")
            outc = cpool.tile([128, 1], F32, tag="outc")

            den3 = cpool.tile([128, 3], F32, tag="den3")
            # kill the 20 dup columns (d=20, i>=20) in i-major exp layout
            exp_im = exp_s[:].rearrange("k (i d) -> k i d", d=NDELTA)
            nc.vector.memset(exp_im[:, 20:40, 19:20], 0.0)
            # scores are O(3) for this model, so exp needs no max-subtraction.
            # exp writes i-major views of exp_s so the g product needs no
            # reordering: col(i, d) = i*20 + (d-1)
            exp_di = exp_s[:].rearrange("k (i d) -> k d i", d=NDELTA)
            nc.scalar.activation(exp_di[:, 0:10, :], sc_h[0][:, 0:SC0],
                                 AF.Exp, accum_out=den3[:, 0:1])
            nc.scalar.activation(exp_di[:, 10:19, :], sc_h[1][:, 0:360],
                                 AF.Exp, accum_out=den3[:, 1:2])
            nc.scalar.activation(
                exp_im[:, 0:20, 19:20],
                sc_h[1][:, 360:380].rearrange("k (i o) -> k i o", o=1),
                AF.Exp, accum_out=den3[:, 2:3])
            # numer = sum_p exp * g  (both i-major now)
            nc.vector.tensor_mul(junk[:], exp_s[:], g_sb[:])
            nc.vector.tensor_reduce(numer[:], junk[:], axis=AXIS.X, op=ALU.add)
            nc.vector.tensor_reduce(denom[:], den3[:], axis=AXIS.X, op=ALU.add)
            nc.vector.reciprocal(rden[:], denom[:])
            nc.vector.tensor_scalar(outc[:], numer[:], rden[:], pb_s[:],
                                    ALU.mult, ALU.add)
            nc.sync.dma_start(out_d[:], outc[:])

    nc.compile()
    return nc


def make_nc(B_c=128, blocks=(8, 8, 16, 32, 32, 32)):
    nc = bacc.Bacc("TRN2", target_bir_lowering=False, debug=False)
    build(nc, B_c=B_c, blocks=blocks)
    return nc


def perm_for(B_c=128, blocks=(8, 8, 16, 32, 32, 32)):
    """perm[slot] = global b stored at SBUF slot."""
    grp_count = [0, 0, 0, 0]
    perm = []
    for nb in blocks:
        for k in range(nb):
            j = k % 4
            perm.append(32 * j + grp_count[j])
            grp_count[j] += 1
    return np.array(perm, np.int64)


def host_prep_consts(attn_w_w, attn_w_b, attn_h_w, attn_h_b, attn_p_w, attn_p_b):
    wT = np.ascontiguousarray(attn_w_w.T).astype(np.float16)
    bias = attn_w_b.reshape(128, 1).astype(np.float32)
    negb = (-attn_w_b).reshape(128, 1).astype(np.float32)
    Zh = np.zeros((128, 64), np.float16)
    Zh[:, 32] = attn_h_w[0].astype(np.float16)
    pb = np.full((128, 1), np.float32(attn_p_b[0]), np.float32)
    return {"wT": wT, "bias": bias, "negb": negb, "Zh": Zh, "pb": pb}


def host_prep_x(x_slice, attn_p_w, blocks=(8, 8, 16, 32, 32, 32)):
    # [B_c, F, E] -> two pre-shifted fp16 copies [E, B_c(perm), 60]
    # plus xtil[e, k, f] = x^T[e, k, f] * p_w[e]  (for the Gram matmuls)
    xT32 = x_slice.transpose(2, 0, 1)
    xT32 = xT32[:, perm_for(x_slice.shape[0], blocks), :]
    xT = xT32.astype(np.float16)
    B_c = x_slice.shape[0]
    xa = np.zeros((128, B_c, 60), np.float16)
    xa[:, :, 0:40] = xT
    xa[:, :, 40:60] = xT[:, :, 0:20]
    xb = np.zeros((128, B_c, 60), np.float16)
    xb[:, :, 0:59] = xa[:, :, 1:60]
    xtil = (xT32 * attn_p_w[0][:, None, None]).astype(np.float16)
    return (np.ascontiguousarray(xa), np.ascontiguousarray(xb),
            np.ascontiguousarray(xtil))


_NC_CACHE = {}
_BLOCKS = (8, 8, 16, 32, 32, 32)


def _get_nc():
    key = _BLOCKS
    if key not in _NC_CACHE:
        _NC_CACHE[key] = make_nc(B_c=128, blocks=key)
    return _NC_CACHE[key]


def kernel(x, attn_w_w, attn_w_b, attn_h_w, attn_h_b, attn_p_w, attn_p_b,
           _trace=False):
    from concourse.bass_utils import run_bass_kernel_spmd
    x = np.asarray(x, np.float32)
    attn_p_w = np.asarray(attn_p_w, np.float32)
    consts = host_prep_consts(np.asarray(attn_w_w), np.asarray(attn_w_b),
                              np.asarray(attn_h_w), np.asarray(attn_h_b),
                              attn_p_w, np.asarray(attn_p_b))
    in_maps = []
    for c in range(8):
        m = dict(consts)
        m["xTa"], m["xTb"], m["xtil"] = host_prep_x(
            x[128 * c:128 * (c + 1)], attn_p_w, blocks=_BLOCKS)
        in_maps.append(m)
    nc = _get_nc()
    res = run_bass_kernel_spmd(nc, in_maps, list(range(8)), trace=_trace)
    out = np.concatenate([res.results[c]["out"][:, 0] for c in range(8)])
    if _trace:
        return out.astype(np.float32), res
    return out.astype(np.float32)
